# revision 1
# baseline (speedup 1.0000x reference)
"""Causal multi-head attention (B=2, S=2048, D=2048, H=16, Dh=128) on 8 NeuronCores.

Sharding: 8 cores = 2 batches x 4 head-groups. Each core handles one batch
element and 4 heads (Dh=128 each):
  - projects q,k,v against its 512-column slice of wq/wk/wv,
  - runs causal attention for its 4 heads,
  - multiplies by its 512-row slice of wo, producing a partial [S, D] output.
Host sums the 4 partial outputs per batch element.

On-device layout notes:
  - Matmul contracts over the partition dim, so activations are kept
    "feature-major": host supplies q/k/v transposed ([D, S]).
  - Scores are computed transposed (scoresT[sk, sq]) so that after exp the
    tile is directly the rhs needed for the PV matmul -- no on-chip
    transposes anywhere.
  - Softmax skips the max-subtraction (scores are ~N(0,1); exp cannot
    overflow) and the denominator is accumulated with a ones-vector matmul.
    The 1/denom scale is applied on the PV output via a DMA partition
    broadcast + vector multiply.
  - All matmuls run in float32r (full fp32 storage; ~4x faster than plain
    fp32 on the PE when the moving free dim is >= 256).
  - The attention inner loop is software-pipelined one step (score matmul
    for t issued before the PV/denom matmuls for t-1) so the PE never waits
    on the exp/mask chain.
  - Attention output (oT) reuses xqT's SBUF: chunk j of xqT[h] is dead once
    chunk j's scores are done, which is exactly when oT[h] chunk j is
    written.
"""

import math

import numpy as np

import concourse.bass as bass
import concourse.tile as tile
from concourse import bacc, mybir
from concourse.bass_utils import run_bass_kernel_spmd

F32 = mybir.dt.float32
F32R = mybir.dt.float32r

N_HEADS_PER_CORE = 4
DH = 128
P = 128

# column offset of the computed region for a diagonal block at offset d
# (d = k_tile - 4*j); capped at 256 so the fp32r matmul keeps >=256 moving
# columns (below that it drops to 1/4 rate and saves nothing).
DIAG_C0 = (0, 128, 256, 256)


def build_nc(S=2048, D=2048, n_heads=N_HEADS_PER_CORE, n_iters=1, phases=5, dup_mm=False, dup_dma=False, final_il=False, in_dt=F32R, stream_bufs=5):
    """Build the per-core Bass program. Every core runs this same NEFF."""
    HD = n_heads * DH  # head-group width (columns of wq/wk/wv, rows of wo)
    SD_CH = D // P     # contraction chunks for the projections
    NQ = S // 512      # 512-wide sequence chunks
    NT = S // P        # 128-row sequence tiles
    ND = D // 512      # 512-wide model-dim chunks of the output

    nc = bacc.Bacc("TRN2", target_bir_lowering=False, debug=False)

    qT = nc.dram_tensor("qT", [D, S], in_dt, kind="ExternalInput").ap()
    kT = nc.dram_tensor("kT", [D, S], in_dt, kind="ExternalInput").ap()
    vT = nc.dram_tensor("vT", [D, S], in_dt, kind="ExternalInput").ap()
    wq = nc.dram_tensor("wq", [D, HD], in_dt, kind="ExternalInput").ap()
    wk = nc.dram_tensor("wk", [D, HD], in_dt, kind="ExternalInput").ap()
    wv = nc.dram_tensor("wv", [D, HD], in_dt, kind="ExternalInput").ap()
    wo = nc.dram_tensor("wo", [HD, D], F32R, kind="ExternalInput").ap()
    cmask = nc.dram_tensor("cmask", [P, 4, 512], F32, kind="ExternalInput").ap()
    out = nc.dram_tensor("out", [S, D], F32, kind="ExternalOutput").ap()

    qT_r = qT.rearrange("(o p) s -> p o s", p=P)
    kT_r = kT.rearrange("(o p) s -> p o s", p=P)
    vT_r = vT.rearrange("(o p) s -> p o s", p=P)
    wq_r = wq.rearrange("(o p) f -> p o f", p=P)
    wk_r = wk.rearrange("(o p) f -> p o f", p=P)
    wv_r = wv.rearrange("(o p) f -> p o f", p=P)
    wo_r = wo.rearrange("(h p) f -> p h f", p=P)
    out_r = out.rearrange("(t p) d -> p t d", p=P)

    inv_sqrt_dh = 1.0 / math.sqrt(DH)

    with tile.TileContext(nc) as tc:
        with (
            tc.tile_pool(name="psum", bufs=8, space="PSUM") as psum,
            tc.tile_pool(name="wpool", bufs=2) as wpool,
            tc.tile_pool(name="bigs", bufs=1) as bigs,
            tc.tile_pool(name="stream", bufs=stream_bufs) as stream,
            tc.tile_pool(name="ptpool", bufs=3) as ptpool,
            tc.tile_pool(name="small", bufs=2) as small,
            tc.tile_pool(name="ostage", bufs=3) as ostage,
            tc.tile_pool(name="consts", bufs=1) as consts,
            tc.tile_pool(name="dram", bufs=2, space="DRAM") as drampool,
        ):
            import contextlib
            loop = tc.For_i(0, n_iters, 1) if n_iters > 1 else contextlib.nullcontext()
            with loop:
                # constants
                ones_f32 = consts.tile([P, 1], F32)
                nc.vector.memset(ones_f32, 1.0)
                ones = consts.tile([P, 1], F32R)
                nc.vector.tensor_copy(ones, ones_f32)
                cm = consts.tile([P, 4, 512], F32)
                nc.gpsimd.dma_start(cm, cmask)

                # persistent activations (feature-major, per head)
                xqT = [bigs.tile([P, S], F32R, name=f"xqT{h}") for h in range(n_heads)]
                xkT = [bigs.tile([P, S], F32R, name=f"xkT{h}") for h in range(n_heads)]
                xv = bigs.tile([P, NT, HD], F32R, name="xv")
                oT = xqT  # oT[h] chunk j overwrites xqT[h] chunk j (dead by then)

                # ---- projections: xqT[h] = (q @ wq_h)^T, xkT likewise ----
                for name, src_r, w_r, dstT in (
                    ("q", qT_r, wq_r, xqT),
                    ("k", kT_r, wk_r, xkT),
                )[: max(1, min(phases, 2))]:
                    w_sb = wpool.tile([P, SD_CH, HD], in_dt, tag="w", name=f"w{name}_sb")
                    for j in range(NQ):
                        ps = [
                            psum.tile([P, 512], F32, tag="ps", name=f"ps_{name}{j}_{h}")
                            for h in range(n_heads)
                        ]
                        for o in range(SD_CH):
                            if j == 0:  # weight chunks arrive just-in-time
                                nc.scalar.dma_start(w_sb[:, o, :], w_r[:, o, :])
                            blk = stream.tile([P, 512], in_dt, tag="stream", name=f"{name}blk")
                            dma_eng = nc.sync if o % 2 == 0 else nc.scalar
                            dma_eng.dma_start(blk, src_r[:, o, 512 * j : 512 * (j + 1)])
                            if dup_dma:
                                blk2 = stream.tile([P, 512], in_dt, tag="stream", name=f"{name}blk2")
                                (nc.scalar if o % 2 == 0 else nc.sync).dma_start(
                                    blk2, src_r[:, o, 512 * j : 512 * (j + 1)])
                            for h in range(n_heads):
                                for _dup in range(2 if dup_mm else 1):
                                    nc.tensor.matmul(
                                        ps[h],
                                        w_sb[:, o, DH * h : DH * (h + 1)],
                                        blk,
                                        start=(o == 0 and _dup == 0),
                                        stop=(o == SD_CH - 1 and _dup == (1 if dup_mm else 0)),
                                    )
                        for h in range(n_heads):
                            nc.vector.tensor_copy(dstT[h][:, 512 * j : 512 * (j + 1)], ps[h])

                # ---- projection: xv = v @ wv (natural layout, heads side by side) ----
                wv_sb = wpool.tile([P, SD_CH, HD], in_dt, tag="w")

                def vproj(sg):
                    """xv tiles 4*sg .. 4*sg+4 = v rows [512sg:512(sg+1)] @ wv."""
                    ps = [
                        psum.tile([P, HD], F32, tag="ps", name=f"ps_v{sg}_{st}")
                        for st in range(4)
                    ]
                    for o in range(SD_CH):
                        if sg == 0:  # weight chunks arrive just-in-time
                            nc.scalar.dma_start(wv_sb[:, o, :], wv_r[:, o, :])
                        blk = stream.tile([P, 512], in_dt, tag="stream", name="vblk")
                        dma_eng = nc.sync if o % 2 == 0 else nc.scalar
                        dma_eng.dma_start(blk, vT_r[:, o, 512 * sg : 512 * (sg + 1)])
                        if dup_dma:
                            blk2 = stream.tile([P, 512], in_dt, tag="stream", name="vblk2")
                            (nc.scalar if o % 2 == 0 else nc.sync).dma_start(
                                blk2, vT_r[:, o, 512 * sg : 512 * (sg + 1)])
                        for st in range(4):
                            nc.tensor.matmul(
                                ps[st],
                                blk[:, P * st : P * (st + 1)],
                                wv_sb[:, o, :],
                                start=(o == 0),
                                stop=(o == SD_CH - 1),
                            )
                    for st in range(4):
                        nc.vector.tensor_copy(xv[:, 4 * sg + st, :], ps[st])

                # ---- causal attention, one (head, 512-wide q-chunk) at a time ----
                def make_pt(h, j, t):
                    """score matmul + exp (+ causal mask on diagonal tiles).

                    Returns (pt_tile, c0): pt[:, c0:] holds exp(scores/sqrt(dh))
                    for k-tile t against q-chunk j; columns < c0 are known-zero
                    contributions (fully masked) and simply not computed.
                    """
                    d = t - 4 * j
                    c0 = DIAG_C0[d] if d >= 0 else 0
                    sc = psum.tile([P, 512], F32, tag="ps", name=f"sc{h}_{j}_{t}")
                    nc.tensor.matmul(
                        sc[:, c0:],
                        xkT[h][:, P * t : P * (t + 1)],
                        xqT[h][:, 512 * j + c0 : 512 * (j + 1)],
                        start=True,
                        stop=True,
                    )
                    pt = ptpool.tile([P, 512], F32R, tag="pt", name=f"pt{h}_{j}_{t}")
                    nc.scalar.activation(
                        pt[:, c0:], sc[:, c0:],
                        mybir.ActivationFunctionType.Exp, scale=inv_sqrt_dh,
                    )
                    if d >= 0:  # tile straddles the diagonal: zero sk > sq
                        nc.vector.tensor_mul(pt[:, c0:], pt[:, c0:], cm[:, d, c0:])
                    return pt, c0

                # ---- output projection: out = sum_h oT[h]^T @ wo_h ----
                # emitted round-by-round (final(j) right after attention round j)
                # so the output DMA spreads across the attention phase.
                wo_sb = wpool.tile([P, n_heads, D], F32R, tag="w")
                for hh in range(n_heads):
                    nc.scalar.dma_start(wo_sb[:, hh, :], wo_r[:, hh, :])

                def final_blocks(jj):
                    for ti in range(4 * jj, 4 * (jj + 1)):
                        for dc in range(ND):
                            fp = psum.tile([P, 512], F32, tag="ps", name=f"fp{ti}_{dc}")
                            for h in range(n_heads):
                                nc.tensor.matmul(
                                    fp,
                                    oT[h][:, P * ti : P * (ti + 1)],
                                    wo_sb[:, h, 512 * dc : 512 * (dc + 1)],
                                    start=(h == 0),
                                    stop=(h == n_heads - 1),
                                )
                            st = ostage.tile([P, 512], F32, tag="ostage")
                            nc.vector.tensor_copy(st, fp)
                            nc.sync.dma_start(out_r[:, ti, 512 * dc : 512 * (dc + 1)], st)

                for j in range(NQ):
                    if phases < 3:
                        break
                    vproj(j)  # attention round j needs xv tiles up to 4*j+3
                    for h in range(n_heads):
                        if phases < 4:
                            break
                        pv = psum.tile([P, 512], F32, tag="ps", name=f"pv{h}_{j}")
                        dn = psum.tile([1, 512], F32, tag="ps", name=f"dn{h}_{j}")
                        nkt = 4 * (j + 1)  # causal: only k-tiles at/below diagonal
                        pts = make_pt(h, j, 0)
                        for t in range(nkt):
                            pt, c0 = pts
                            if t + 1 < nkt:  # pipeline: next scores before PV(t)
                                pts = make_pt(h, j, t + 1)
                            nc.tensor.matmul(
                                pv[:, c0:],
                                xv[:, t, DH * h : DH * (h + 1)],
                                pt[:, c0:],
                                start=(t == 0),
                                stop=(t == nkt - 1),
                            )
                            nc.tensor.matmul(
                                dn[:, c0:],
                                ones,
                                pt[:, c0:],
                                start=(t == 0),
                                stop=(t == nkt - 1),
                            )
                        dinv = small.tile([1, 512], F32, tag="dinv")
                        nc.vector.reciprocal(dinv, dn)
                        ddram = drampool.tile([1, 512], F32, tag="ddram")
                        nc.scalar.dma_start(ddram, dinv)
                        db = small.tile([P, 512], F32, tag="db")
                        nc.scalar.dma_start(db, ddram.to_broadcast((P, 512)))
                        nc.vector.tensor_mul(oT[h][:, 512 * j : 512 * (j + 1)], pv, db)
                    if phases >= 5 and final_il:
                        final_blocks(j)

                if phases >= 5 and not final_il:
                    for jj in range(NQ):
                        final_blocks(jj)


    nc.compile()
    return nc


def make_cmask():
    """cmask[sk_local, d, sq_local] = 1 if (128*d + sk_local) <= sq_local."""
    sk = np.arange(P)[:, None, None]
    d = np.arange(4)[None, :, None]
    sq = np.arange(512)[None, None, :]
    return ((P * d + sk) <= sq).astype(np.float32)


def run(q, k, v, wq, wk, wv, wo, trace=False, trace_cores=None, **build_kw):
    B, S, D = q.shape
    n_groups = 4  # head groups; 8 cores = B x n_groups
    HD = D // n_groups
    nc = build_nc(S=S, D=D, **build_kw)
    cast = (lambda a: a)
    if build_kw.get("in_dt") is not None and build_kw["in_dt"] != F32R:
        import ml_dtypes

        cast = (lambda a: np.ascontiguousarray(a).astype(ml_dtypes.bfloat16))

    cmask = make_cmask()
    qT = [np.ascontiguousarray(q[b].T) for b in range(B)]
    kT = [np.ascontiguousarray(k[b].T) for b in range(B)]
    vT = [np.ascontiguousarray(v[b].T) for b in range(B)]

    in_maps = []
    for core in range(8):
        b, g = divmod(core, n_groups)
        in_maps.append(
            {
                "qT": cast(qT[b]),
                "kT": cast(kT[b]),
                "vT": cast(vT[b]),
                "wq": cast(np.ascontiguousarray(wq[:, HD * g : HD * (g + 1)])),
                "wk": cast(np.ascontiguousarray(wk[:, HD * g : HD * (g + 1)])),
                "wv": cast(np.ascontiguousarray(wv[:, HD * g : HD * (g + 1)])),
                "wo": np.ascontiguousarray(wo[HD * g : HD * (g + 1), :]),
                "cmask": cmask,
            }
        )

    res = run_bass_kernel_spmd(
        nc,
        in_maps,
        core_ids=list(range(8)),
        trace=trace,
        **({"trace_cores": trace_cores} if trace_cores else {}),
    )
    parts = [r["out"] for r in res.results]
    full = np.stack(
        [np.add.reduce(parts[b * n_groups : (b + 1) * n_groups]) for b in range(B)]
    ).astype(np.float32)
    return full, res


def kernel(q, k, v, wq, wk, wv, wo):
    full, _ = run(q, k, v, wq, wk, wv, wo)
    return full



# revision 3
# speedup vs baseline: 1.2074x; 1.2074x over previous
"""Causal multi-head attention (B=2, S=2048, D=2048, H=16, Dh=128) on 8 NeuronCores.

Sharding: 8 cores = 2 batches x 4 head-groups. Each core handles one batch
element and 4 heads (Dh=128 each):
  - projects q,k,v against its 512-column slice of wq/wk/wv,
  - runs causal attention for its 4 heads,
  - multiplies by its 512-row slice of wo, producing a partial [S, D] output.
Host sums the 4 partial outputs per batch element (fp32 accumulate).

v2 notes (vs the fp32r baseline):
  - Everything bf16 on the wire and in SBUF; PSUM accumulates fp32. Halves
    HBM traffic and host<->device transfer, and enables FWL weight loads.
  - The whole kernel is a single j-loop over 512-wide query chunks:
    project chunk j (q,k,v) -> attention rounds for chunk j -> output
    projection for chunk j. This gives the scheduler independent PE work
    (projections / wo matmuls) to fill the exp-latency bubbles inside the
    attention rounds, keeping the PE warm (HAM K=8/8).
  - Activations stream in as one 2MB DMA per (tensor, chunk) instead of 16
    small ones.
  - Scores are computed transposed (scoresT[sk, sq]) so the exp'd tile is
    directly the rhs of the PV matmul; softmax denominator accumulates via
    a ones-vector matmul; 1/denom is broadcast across partitions with a
    gpsimd partition_broadcast (no DRAM round-trip).
  - Causal handling at 128 granularity: for a diagonal tile at offset d,
    columns < 128*d are skipped outright and only the single 128x128
    block that straddles the diagonal is masked (one small bf16 multiply).
  - The score->exp->PV chain is pipelined two k-tiles deep.
"""

import math

import ml_dtypes
import numpy as np

import concourse.bass as bass
import concourse.tile as tile
from concourse import bacc, mybir
from concourse.bass_utils import run_bass_kernel_spmd

F32 = mybir.dt.float32
BF16 = mybir.dt.bfloat16

N_HEADS_PER_CORE = 4
DH = 128
P = 128


def build_nc(S=2048, D=2048, n_heads=N_HEADS_PER_CORE, use_pbcast=True,
             pt_ahead=2, stream_bufs=2):
    """Build the per-core Bass program. Every core runs this same NEFF."""
    HD = n_heads * DH  # head-group width (columns of wq/wk/wv, rows of wo)
    SD = D // P        # contraction chunks for the projections
    NQ = S // 512      # 512-wide sequence chunks
    NT = S // P        # 128-row sequence tiles
    ND = D // 512      # 512-wide model-dim chunks of the output

    inv_sqrt_dh = 1.0 / math.sqrt(DH)

    nc = bacc.Bacc("TRN2", target_bir_lowering=False, debug=False)

    qT = nc.dram_tensor("qT", [D, S], BF16, kind="ExternalInput").ap()
    kT = nc.dram_tensor("kT", [D, S], BF16, kind="ExternalInput").ap()
    vT = nc.dram_tensor("vT", [D, S], BF16, kind="ExternalInput").ap()
    wq = nc.dram_tensor("wq", [D, HD], BF16, kind="ExternalInput").ap()
    wk = nc.dram_tensor("wk", [D, HD], BF16, kind="ExternalInput").ap()
    wv = nc.dram_tensor("wv", [D, HD], BF16, kind="ExternalInput").ap()
    wo = nc.dram_tensor("wo", [HD, D], BF16, kind="ExternalInput").ap()
    cmask = nc.dram_tensor("cmask", [P, P], BF16, kind="ExternalInput").ap()
    out = nc.dram_tensor("out", [S, D], BF16, kind="ExternalOutput").ap()

    qT_r = qT.rearrange("(o p) s -> p o s", p=P)
    kT_r = kT.rearrange("(o p) s -> p o s", p=P)
    vT_r = vT.rearrange("(o p) s -> p o s", p=P)
    wq_r = wq.rearrange("(o p) f -> p o f", p=P)
    wk_r = wk.rearrange("(o p) f -> p o f", p=P)
    wv_r = wv.rearrange("(o p) f -> p o f", p=P)
    wo_r = wo.rearrange("(h p) f -> p h f", p=P)
    out_r = out.rearrange("(t p) d -> p t d", p=P)

    with tile.TileContext(nc) as tc:
        with (
            tc.tile_pool(name="consts", bufs=1) as consts,
            tc.tile_pool(name="wpool", bufs=1) as wpool,
            tc.tile_pool(name="bigs", bufs=1) as bigs,
            tc.tile_pool(name="stream", bufs=stream_bufs) as stream,
            tc.tile_pool(name="ptpool", bufs=6) as ptpool,
            tc.tile_pool(name="small", bufs=2) as small,
            tc.tile_pool(name="dbpool", bufs=2) as dbpool,
            tc.tile_pool(name="ostage", bufs=3) as ostage,
            tc.tile_pool(name="pp", bufs=2, space="PSUM") as pp,
            tc.tile_pool(name="scp", bufs=2, space="PSUM") as scp,
            tc.tile_pool(name="pvp", bufs=2, space="PSUM") as pvp,
            tc.tile_pool(name="dnp", bufs=2, space="PSUM") as dnp,
            tc.tile_pool(name="dram", bufs=2, space="DRAM") as drampool,
        ):
            ones = consts.tile([P, 1], BF16)
            nc.vector.memset(ones, 1.0)
            cm = consts.tile([P, P], BF16)
            nc.scalar.dma_start(cm, cmask)

            # whole weight set prefetched up front (8MB bf16 total)
            wq_sb = wpool.tile([P, SD, HD], BF16, name="wq_sb")
            nc.scalar.dma_start(wq_sb, wq_r)
            wk_sb = wpool.tile([P, SD, HD], BF16, name="wk_sb")
            nc.sync.dma_start(wk_sb, wk_r)
            wv_sb = wpool.tile([P, SD, HD], BF16, name="wv_sb")
            nc.scalar.dma_start(wv_sb, wv_r)
            wo_sb = wpool.tile([P, n_heads, D], BF16, name="wo_sb")
            nc.sync.dma_start(wo_sb, wo_r)

            # persistent activations (feature-major, per head)
            xqT = [bigs.tile([P, S], BF16, name=f"xqT{h}") for h in range(n_heads)]
            xkT = [bigs.tile([P, S], BF16, name=f"xkT{h}") for h in range(n_heads)]
            xv = bigs.tile([P, NT, HD], BF16, name="xv")
            oT = [bigs.tile([P, S], BF16, name=f"oT{h}") for h in range(n_heads)]

            for j in range(NQ):
                sl = slice(512 * j, 512 * (j + 1))

                # ---- stream in chunk j of q, k, v (one 2MB DMA each) ----
                qb = stream.tile([P, SD, 512], BF16, tag="blk", name="qb")
                nc.sync.dma_start(qb, qT_r[:, :, sl])
                kb = stream.tile([P, SD, 512], BF16, tag="blk", name="kb")
                nc.scalar.dma_start(kb, kT_r[:, :, sl])
                vb = stream.tile([P, SD, 512], BF16, tag="blk", name="vb")
                nc.sync.dma_start(vb, vT_r[:, :, sl])

                # ---- projections for chunk j ----
                for h in range(n_heads):
                    ps = pp.tile([P, 512], F32, tag="pp", name=f"psq{j}_{h}")
                    for o in range(SD):
                        nc.tensor.matmul(
                            ps, wq_sb[:, o, DH * h : DH * (h + 1)], qb[:, o, :],
                            start=(o == 0), stop=(o == SD - 1),
                        )
                    nc.vector.tensor_copy(xqT[h][:, sl], ps)
                for h in range(n_heads):
                    ps = pp.tile([P, 512], F32, tag="pp", name=f"psk{j}_{h}")
                    for o in range(SD):
                        nc.tensor.matmul(
                            ps, wk_sb[:, o, DH * h : DH * (h + 1)], kb[:, o, :],
                            start=(o == 0), stop=(o == SD - 1),
                        )
                    nc.vector.tensor_copy(xkT[h][:, sl], ps)
                for st in range(4):
                    ps = pp.tile([P, HD], F32, tag="pp", name=f"psv{j}_{st}")
                    for o in range(SD):
                        nc.tensor.matmul(
                            ps, vb[:, o, P * st : P * (st + 1)], wv_sb[:, o, :],
                            start=(o == 0), stop=(o == SD - 1),
                        )
                    nc.vector.tensor_copy(xv[:, 4 * j + st, :], ps)

                # ---- causal attention for chunk j, one head at a time ----
                for h in range(n_heads):
                    nkt = 4 * (j + 1)  # causal: only k-tiles at/below diagonal
                    pv = pvp.tile([P, 512], F32, tag="pv", name=f"pv{j}_{h}")
                    dn = dnp.tile([1, 512], F32, tag="dn", name=f"dn{j}_{h}")

                    def make_pt(t, h=h, j=j):
                        """score matmul + exp (+ 128x128 diagonal-block mask).

                        Returns (pt, c0): pt[:, c0:] = exp(scoresT/sqrt(dh)) for
                        k-tile t vs q-chunk j; columns < c0 are fully masked and
                        not computed.
                        """
                        d = t - 4 * j
                        c0 = P * d if d > 0 else 0
                        sc = scp.tile([P, 512], F32, tag="sc", name=f"sc{j}_{h}_{t}")
                        nc.tensor.matmul(
                            sc[:, c0:],
                            xkT[h][:, P * t : P * (t + 1)],
                            xqT[h][:, 512 * j + c0 : 512 * (j + 1)],
                            start=True, stop=True,
                        )
                        pt = ptpool.tile([P, 512], BF16, tag="pt", name=f"pt{j}_{h}_{t}")
                        nc.scalar.activation(
                            pt[:, c0:], sc[:, c0:],
                            mybir.ActivationFunctionType.Exp, scale=inv_sqrt_dh,
                        )
                        if d >= 0:  # mask the single block straddling the diagonal
                            nc.vector.tensor_mul(
                                pt[:, c0 : c0 + P], pt[:, c0 : c0 + P], cm
                            )
                        return pt, c0

                    pts = [make_pt(0)]
                    if nkt > 1:
                        pts.append(make_pt(1))
                    for t in range(nkt):
                        pt, c0 = pts[t]
                        if t + pt_ahead < nkt:
                            pts.append(make_pt(t + pt_ahead))
                        nc.tensor.matmul(
                            pv[:, c0:],
                            xv[:, t, DH * h : DH * (h + 1)],
                            pt[:, c0:],
                            start=(t == 0), stop=(t == nkt - 1),
                        )
                        nc.tensor.matmul(
                            dn[:, c0:], ones, pt[:, c0:],
                            start=(t == 0), stop=(t == nkt - 1),
                        )

                    dinv = small.tile([1, 512], F32, tag="dinv")
                    nc.vector.reciprocal(dinv, dn)
                    db = dbpool.tile([P, 512], F32, tag="db")
                    if use_pbcast:
                        nc.gpsimd.partition_broadcast(db, dinv)
                    else:
                        ddram = drampool.tile([1, 512], F32, tag="ddram")
                        nc.scalar.dma_start(ddram, dinv)
                        nc.scalar.dma_start(db, ddram.to_broadcast((P, 512)))
                    nc.vector.tensor_mul(oT[h][:, sl], pv, db)

                # ---- output projection for chunk j ----
                for ti in range(4 * j, 4 * (j + 1)):
                    for dc in range(ND):
                        fp = pp.tile([P, 512], F32, tag="pp", name=f"fp{ti}_{dc}")
                        for h in range(n_heads):
                            nc.tensor.matmul(
                                fp,
                                oT[h][:, P * ti : P * (ti + 1)],
                                wo_sb[:, h, 512 * dc : 512 * (dc + 1)],
                                start=(h == 0), stop=(h == n_heads - 1),
                            )
                        stg = ostage.tile([P, 512], BF16, tag="ostage")
                        nc.vector.tensor_copy(stg, fp)
                        nc.sync.dma_start(out_r[:, ti, 512 * dc : 512 * (dc + 1)], stg)

    nc.compile()
    return nc


def make_cmask():
    """cmask[sk_local, sq_local] = 1 if sk_local <= sq_local (bf16)."""
    return np.triu(np.ones((P, P), np.float32)).astype(ml_dtypes.bfloat16)


def run(q, k, v, wq, wk, wv, wo, trace=False, trace_cores=None, **build_kw):
    B, S, D = q.shape
    n_groups = 4  # head groups; 8 cores = B x n_groups
    HD = D // n_groups
    nc = build_nc(S=S, D=D, **build_kw)
    bf = ml_dtypes.bfloat16

    cmask = make_cmask()
    qT = [np.ascontiguousarray(q[b].T).astype(bf) for b in range(B)]
    kT = [np.ascontiguousarray(k[b].T).astype(bf) for b in range(B)]
    vT = [np.ascontiguousarray(v[b].T).astype(bf) for b in range(B)]

    in_maps = []
    for core in range(8):
        b, g = divmod(core, n_groups)
        in_maps.append(
            {
                "qT": qT[b],
                "kT": kT[b],
                "vT": vT[b],
                "wq": np.ascontiguousarray(wq[:, HD * g : HD * (g + 1)]).astype(bf),
                "wk": np.ascontiguousarray(wk[:, HD * g : HD * (g + 1)]).astype(bf),
                "wv": np.ascontiguousarray(wv[:, HD * g : HD * (g + 1)]).astype(bf),
                "wo": np.ascontiguousarray(wo[HD * g : HD * (g + 1), :]).astype(bf),
                "cmask": cmask,
            }
        )

    res = run_bass_kernel_spmd(
        nc,
        in_maps,
        core_ids=list(range(8)),
        trace=trace,
        **({"trace_cores": trace_cores} if trace_cores else {}),
    )
    parts = [r["out"].astype(np.float32) for r in res.results]
    full = np.stack(
        [np.add.reduce(parts[b * n_groups : (b + 1) * n_groups]) for b in range(B)]
    )
    return full, res


def kernel(q, k, v, wq, wk, wv, wo):
    full, _ = run(q, k, v, wq, wk, wv, wo)
    return full
